# revision 11
# baseline (speedup 1.0000x reference)
"""Trainium2 Bass kernel for ColorizationLoss (MSE + 0.2*VGG-perceptual + 0.2*SSIM).

Sharding: 8 cores; core c handles batch b=c//2, row-half h=c%2 (rows
[256h, 256h+256) of the 512-row image) for BOTH pred and target streams.
Each core gets a 268-row slab (halo 6 rows each side, zero-padded at true
image edges, matching SAME conv / SSIM zero padding). Per-core partial
sums are returned as [128, 24] per-partition accumulators; the host
combines them into the scalar loss.
"""
import sys

sys.path.insert(0, "/opt/trn_rl_repo")

import numpy as np
import concourse.bass as bass
import concourse.tile as tile
from concourse import mybir
from concourse.bass_utils import run_bass_kernel_spmd
from concourse.vector_clock import ScopedClock

F32 = mybir.dt.float32
F32R = mybir.dt.float32r
OP = mybir.AluOpType
ACT = mybir.ActivationFunctionType

# ---------------------------------------------------------------------------
# Walrus in this container caps sync waits at 1 per non-EventSemaphore
# instruction; TileContext's exit drain piles every global-clock wait onto a
# single Drain. Spill the extras onto standalone NOPs.
def _patched_drain_and_barrier(self, tick_clock, wait_clock):
    nc = self.nc
    drain_inst = nc.sync.drain()
    wait_clock.add_sem_waits(
        drain_inst.ins, ScopedClock({None: tick_clock.global_clock})
    )
    si = drain_inst.ins.sync_info
    if si is not None and len(si.on_wait) > 1:
        extras = list(si.on_wait[1:])
        drain_inst.ins.sync_info = mybir.SyncInfo(
            on_wait=[si.on_wait[0]], on_update=list(si.on_update)
        )
        for w in extras:
            nop = nc.sync.add_instruction(
                mybir.InstNoOp(name=nc.get_next_instruction_name(), ins=[], outs=[])
            )
            nop.ins.sync_info = mybir.SyncInfo(on_wait=[w], on_update=[])
    nc.all_engine_barrier()
    popped = nc._tile_sem_poison_stack.pop()
    assert popped is self._sem_poison
    nc.clear_and_free_semaphores(list(self.sems.allocated().values()))
    nc.all_engine_barrier()


tile.TileContext._drain_and_barrier = _patched_drain_and_barrier


def _legalize_waits(nc):
    """Split multi-wait instructions: this walrus caps sync waits at 1 per
    instruction (2 for EventSemaphore). Hoist extras onto same-engine NOPs
    inserted immediately before the instruction."""
    for f in nc.m.functions:
        for bb in f.blocks:
            new = []
            for inst in bb.instructions:
                cap = 2 if isinstance(inst, mybir.InstEventSemaphore) else 1
                si = inst.sync_info
                if si is not None and len(si.on_wait) > cap:
                    extras = list(si.on_wait[: len(si.on_wait) - cap])
                    keep = list(si.on_wait[len(si.on_wait) - cap :])
                    inst.sync_info = mybir.SyncInfo(
                        on_wait=keep, on_update=list(si.on_update)
                    )
                    for w in extras:
                        nop = mybir.InstNoOp(
                            name=nc.get_next_instruction_name(), ins=[], outs=[]
                        )
                        nop.engine = inst.engine
                        nop.sync_info = mybir.SyncInfo(on_wait=[w], on_update=[])
                        new.append(nop)
                new.append(inst)
            bb.instructions[:] = new

# ---------------------------------------------------------------------------
# Geometry (slab coords: slab row r == global row 256*h - 6 + r)
RSLAB = 268          # slab rows at 512 res
W = 512
WP = W + 2           # padded width at 512 res
W2 = 256             # 256-res width
W2P = W2 + 2

# fixed constants of the loss
C1 = 0.01 ** 2
C2 = 0.03 ** 2
INV121 = 1.0 / 121.0
INV121SQ = 1.0 / (121.0 * 121.0)
DELTA = 6.0 / 29.0
FY0 = 16.0 / 116.0
LIN_A = 3.0 * DELTA * DELTA      # finv linear slope
LIN_B = 4.0 / 29.0               # finv linear offset
# rgb = M @ [X, Z] with Y == 0 (L == 0 everywhere)
CXR = 3.240479 * 0.950456
CZR = -0.498535 * 1.088754
CXG = -0.969256 * 0.950456
CZG = 0.041556 * 1.088754
CXB = 0.055648 * 0.950456
CZB = 1.057311 * 1.088754

N_SLOTS = 24
SLOT_MSE = 0         # 2 slots
SLOT_SSIM = 2        # 4 slots
SLOT_PERC = 6        # 16 slots


def _bands(start, end, bh):
    out = []
    a = start
    while a < end:
        out.append((a, min(bh, end - a)))
        a += bh
    return out


def build_program():
    nc = bass.Bass()

    pred = nc.declare_dram_parameter("pred", [2, RSLAB, W], F32, isOutput=False)
    targ = nc.declare_dram_parameter("targ", [2, RSLAB, W], F32, isOutput=False)
    w1i = nc.declare_dram_parameter("w1i", [27, 64], F32R, isOutput=False)
    w2t = nc.declare_dram_parameter("w2t", [9, 64, 64], F32R, isOutput=False)
    w3t = nc.declare_dram_parameter("w3t", [9, 64, 128], F32R, isOutput=False)
    w4t = nc.declare_dram_parameter("w4t", [9, 128, 128], F32R, isOutput=False)
    b1d = nc.declare_dram_parameter("b1d", [128, 1], F32, isOutput=False)
    b2d = nc.declare_dram_parameter("b2d", [128, 1], F32, isOutput=False)
    b3d = nc.declare_dram_parameter("b3d", [128, 1], F32, isOutput=False)
    bandA = nc.declare_dram_parameter("bandA", [128, 128], F32, isOutput=False)
    bandB = nc.declare_dram_parameter("bandB", [128, 128], F32, isOutput=False)
    parts_out = nc.declare_dram_parameter("parts", [128, N_SLOTS], F32, isOutput=True)

    # per-stream HBM scratch
    scr = {}
    for s in ("P", "T"):
        scr[s] = dict(
            rgb=nc.dram_tensor(f"rgb{s}", [3, RSLAB, WP], F32R),
            c1=nc.dram_tensor(f"c1{s}", [64, RSLAB, WP], F32R),
            c2=nc.dram_tensor(f"c2{s}", [64, RSLAB, W], F32),
            pl=nc.dram_tensor(f"pl{s}", [64, 134, W2P], F32R),
            c3=nc.dram_tensor(f"c3{s}", [128, 134, W2P], F32R),
        )
    pf = nc.dram_tensor("pf", [128, 128, W2], F32)

    ins = {"pred": pred, "targ": targ}

    with tile.TileContext(nc) as tc:
        with tc.tile_pool(name="const", bufs=1) as cpool:
            # constants in SBUF
            w1s = cpool.tile([27, 64], F32R)
            nc.sync.dma_start(w1s[:], w1i[:])
            w2s = cpool.tile([64, 9, 64], F32R)
            nc.sync.dma_start(w2s[:], w2t[:].rearrange("t c o -> c t o"))
            w3s = cpool.tile([64, 9, 128], F32R)
            nc.sync.dma_start(w3s[:], w3t[:].rearrange("t c o -> c t o"))
            w4s = cpool.tile([128, 9, 128], F32R)
            nc.sync.dma_start(w4s[:], w4t[:].rearrange("t c o -> c t o"))
            b1s = cpool.tile([128, 1], F32)
            nc.sync.dma_start(b1s[:], b1d[:])
            b2s = cpool.tile([128, 1], F32)
            nc.sync.dma_start(b2s[:], b2d[:])
            b3s = cpool.tile([128, 1], F32)
            nc.sync.dma_start(b3s[:], b3d[:])
            bAs = cpool.tile([128, 128], F32)
            nc.sync.dma_start(bAs[:], bandA[:])
            bBs = cpool.tile([128, 128], F32)
            nc.sync.dma_start(bBs[:], bandB[:])
            zt = cpool.tile([128, 536], F32R)
            nc.vector.memset(zt[:].bitcast(F32), 0.0)
            acc = cpool.tile([128, N_SLOTS], F32)
            nc.vector.memset(acc[:], 0.0)

            # zero the padding columns of the padded scratch tensors
            for s in ("P", "T"):
                for t_, ch, nr, wp in (
                    (scr[s]["rgb"], 3, RSLAB, WP),
                    (scr[s]["c1"], 64, RSLAB, WP),
                    (scr[s]["pl"], 64, 134, W2P),
                    (scr[s]["c3"], 128, 134, W2P),
                ):
                    for col in (0, wp - 1):
                        nc.sync.dma_start(t_[:, :, col : col + 1], zt[0:ch, 0:nr])

            # =========== per-stream VGG pipeline ===========
            for s, src in (("P", pred), ("T", targ)):
                sc = scr[s]
                # ---- LAB -> RGB (pixel-major tiles) ----
                with tc.tile_pool(name="lab", bufs=2) as lp:
                    for tr0, tnr in ((0, 128), (128, 128), (256, 12)):
                        ab = lp.tile([tnr, 2, W], F32, tag="ab")
                        nc.sync.dma_start(
                            ab[:], src[:, tr0 : tr0 + tnr, :].rearrange("c r w -> r c w")
                        )
                        a_ = ab[:, 0, :]
                        b_ = ab[:, 1, :]
                        fv = {}
                        for name, chan, c0 in (("x", a_, 128.0 / 500.0), ("z", b_, -128.0 / 200.0)):
                            f = lp.tile([tnr, W], F32, tag=f"f{name}")
                            nc.vector.tensor_scalar(f[:], chan, c0, FY0, OP.mult, OP.add)
                            cube = lp.tile([tnr, W], F32, tag=f"cube{name}")
                            nc.vector.tensor_mul(cube[:], f[:], f[:])
                            nc.vector.tensor_mul(cube[:], cube[:], f[:])
                            lin = lp.tile([tnr, W], F32, tag=f"lin{name}")
                            nc.vector.tensor_scalar(lin[:], f[:], LIN_B, LIN_A, OP.subtract, OP.mult)
                            mask = lp.tile([tnr, W], mybir.dt.int8, tag=f"mask{name}")
                            nc.vector.tensor_single_scalar(mask[:], f[:], DELTA, OP.is_gt)
                            v = lp.tile([tnr, W], F32, tag=f"v{name}")
                            nc.vector.select(v[:], mask[:], cube[:], lin[:])
                            fv[name] = v
                        rgb = lp.tile([tnr, 3, W], F32, tag="rgb")
                        for ci, (cx, cz) in enumerate(((CXR, CZR), (CXG, CZG), (CXB, CZB))):
                            tmp = lp.tile([tnr, W], F32, tag="ztmp")
                            nc.vector.tensor_scalar_mul(tmp[:], fv["z"][:], cz)
                            chn = lp.tile([tnr, W], F32, tag="chn")
                            nc.vector.scalar_tensor_tensor(chn[:], fv["x"][:], cx, tmp[:], OP.mult, OP.add)
                            # clip to [1e-8, 1]
                            nc.vector.tensor_scalar(chn[:], chn[:], 1e-8, 1.0, OP.max, OP.min)
                            maskg = lp.tile([tnr, W], mybir.dt.int8, tag="maskg")
                            nc.vector.tensor_single_scalar(maskg[:], chn[:], 0.0031308, OP.is_le)
                            ling = lp.tile([tnr, W], F32, tag="ling")
                            nc.vector.tensor_scalar_mul(ling[:], chn[:], 12.92)
                            lnv = lp.tile([tnr, W], F32, tag="lnv")
                            nc.scalar.activation(lnv[:], chn[:], ACT.Ln)
                            pw = lp.tile([tnr, W], F32, tag="pw")
                            nc.scalar.activation(pw[:], lnv[:], ACT.Exp, scale=1.0 / 2.4)
                            nc.vector.tensor_scalar(pw[:], pw[:], 1.055, 0.055, OP.mult, OP.subtract)
                            nc.vector.select(rgb[:, ci, :], maskg[:], ling[:], pw[:])
                        nc.sync.dma_start(
                            sc["rgb"][:, tr0 : tr0 + tnr, 1 : 1 + W].rearrange("c r w -> r c w").bitcast(F32),
                            rgb[:],
                        )

                # ---- conv1: K=27 im2col, 1 matmul per output row ----
                with (
                    tc.tile_pool(name="cv1", bufs=2) as p1,
                    tc.tile_pool(name="ps1", bufs=2, space="PSUM") as pp1,
                ):
                    rgbt = sc["rgb"]
                    for a, B in _bands(1, 267, 16):
                        im = p1.tile([27, B, W], F32R, tag="im")
                        for c in range(3):
                            for dy in range(3):
                                src_ap = bass.AP(
                                    rgbt[:].tensor,
                                    c * (RSLAB * WP) + (a - 1 + dy) * WP,
                                    [[1, 3], [WP, B], [1, W]],
                                )
                                p0 = 9 * c + 3 * dy
                                nc.sync.dma_start(im[p0 : p0 + 3, :, :], src_ap)
                        obuf = p1.tile([64, B, W], F32R, tag="obuf")
                        for j0 in range(0, B, 4):
                            g = min(4, B - j0)
                            ps = pp1.tile([64, 4, W], F32, tag="ps")
                            for j in range(j0, j0 + g):
                                nc.tensor.matmul(
                                    ps[:, j - j0, :],
                                    w1s[:],
                                    im[:, j, :],
                                    start=True,
                                    stop=True,
                                )
                            nc.scalar.activation(
                                obuf[:, j0 : j0 + g, :].rearrange("p r w -> p (r w)"),
                                ps[:, 0:g, :].rearrange("p r w -> p (r w)"),
                                ACT.Relu,
                                bias=b1s[0:64, :],
                            )
                        nc.sync.dma_start(sc["c1"][:, a : a + B, 1 : 1 + W], obuf[:])

                # ---- conv2: K=64, 9 taps ----
                with (
                    tc.tile_pool(name="cv2", bufs=2) as p2,
                    tc.tile_pool(name="ps2", bufs=2, space="PSUM") as pp2,
                ):
                    for a, B in _bands(2, 266, 16):
                        inb = p2.tile([64, B + 2, WP], F32R, tag="inb")
                        nc.sync.dma_start(inb[:], sc["c1"][:, a - 1 : a + B + 1, :])
                        obuf = p2.tile([64, B, W], F32, tag="obuf2")
                        for j0 in range(0, B, 4):
                            g = min(4, B - j0)
                            ps = pp2.tile([64, 4, W], F32, tag="ps2")
                            for j in range(j0, j0 + g):
                                for t9 in range(9):
                                    dy, dx = divmod(t9, 3)
                                    nc.tensor.matmul(
                                        ps[:, j - j0, :],
                                        w2s[:, t9, :],
                                        inb[:, j + dy, dx : dx + W],
                                        start=(t9 == 0),
                                        stop=(t9 == 8),
                                    )
                            nc.scalar.activation(
                                obuf[:, j0 : j0 + g, :].rearrange("p r w -> p (r w)"),
                                ps[:, 0:g, :].rearrange("p r w -> p (r w)"),
                                ACT.Relu,
                                bias=b2s[0:64, :],
                            )
                        nc.sync.dma_start(sc["c2"][:, a : a + B, :], obuf[:])

                # ---- maxpool 2x2 ----
                with tc.tile_pool(name="mp", bufs=2) as pm:
                    for a, B in _bands(1, 133, 8):
                        inb = pm.tile([64, 2 * B, W], F32, tag="mpin")
                        nc.sync.dma_start(inb[:], sc["c2"][:, 2 * a : 2 * a + 2 * B, :])
                        rview = inb[:].rearrange("c (r two) w -> c r two w", two=2)
                        tmp = pm.tile([64, B, W], F32, tag="mptmp")
                        nc.vector.tensor_max(tmp[:], rview[:, :, 0, :], rview[:, :, 1, :])
                        cview = tmp[:].rearrange("c r (x two) -> c r x two", two=2)
                        ob = pm.tile([64, B, W2], F32R, tag="mpout")
                        nc.vector.tensor_max(ob[:], cview[:, :, :, 0], cview[:, :, :, 1])
                        nc.sync.dma_start(sc["pl"][:, a : a + B, 1 : 1 + W2], ob[:])

                # ---- conv3: K=64 -> 128, 2 rows per matmul ----
                with (
                    tc.tile_pool(name="cv3", bufs=2) as p3,
                    tc.tile_pool(name="ps3", bufs=2, space="PSUM") as pp3,
                ):
                    for a, B in _bands(2, 132, 16):
                        inb = p3.tile([64, B + 2, W2P], F32R, tag="inb3")
                        nc.sync.dma_start(inb[:], sc["pl"][:, a - 1 : a + B + 1, :])
                        obuf = p3.tile([128, B, W2], F32R, tag="obuf3")
                        npair = B // 2
                        for g0 in range(0, npair, 4):
                            ng = min(4, npair - g0)
                            ps = pp3.tile([128, 4, 2, W2], F32, tag="ps3")
                            for g in range(g0, g0 + ng):
                                for t9 in range(9):
                                    dy, dx = divmod(t9, 3)
                                    nc.tensor.matmul(
                                        ps[:, g - g0, :, :],
                                        w3s[:, t9, :],
                                        inb[:, 2 * g + dy : 2 * g + dy + 2, dx : dx + W2],
                                        start=(t9 == 0),
                                        stop=(t9 == 8),
                                    )
                            nc.scalar.activation(
                                obuf[:, 2 * g0 : 2 * g0 + 2 * ng, :].rearrange("p r w -> p (r w)"),
                                ps[:, 0:ng, :, :].rearrange("p r two w -> p (r two w)"),
                                ACT.Relu,
                                bias=b3s[:],
                            )
                        nc.sync.dma_start(sc["c3"][:, a : a + B, 1 : 1 + W2], obuf[:])

                # ---- conv4 (+ store pf / diff-square-accumulate) ----
                with (
                    tc.tile_pool(name="cv4", bufs=2) as p4,
                    tc.tile_pool(name="ps4", bufs=2, space="PSUM") as pp4,
                ):
                    for bi, (a, B) in enumerate(_bands(3, 131, 16)):
                        inb = p4.tile([128, B + 2, W2P], F32R, tag="inb4")
                        nc.sync.dma_start(inb[:], sc["c3"][:, a - 1 : a + B + 1, :])
                        if s == "P":
                            pfb = p4.tile([128, B, W2], F32, tag="pfb")
                        else:
                            pfb = p4.tile([128, B, W2], F32, tag="pfb")
                            nc.sync.dma_start(pfb[:], pf[:, a - 3 : a - 3 + B, :])
                        npair = B // 2
                        for g0 in range(0, npair, 4):
                            ng = min(4, npair - g0)
                            ps = pp4.tile([128, 4, 2, W2], F32, tag="ps4")
                            for g in range(g0, g0 + ng):
                                for t9 in range(9):
                                    dy, dx = divmod(t9, 3)
                                    nc.tensor.matmul(
                                        ps[:, g - g0, :, :],
                                        w4s[:, t9, :],
                                        inb[:, 2 * g + dy : 2 * g + dy + 2, dx : dx + W2],
                                        start=(t9 == 0),
                                        stop=(t9 == 8),
                                    )
                            ps_flat = ps[:, 0:ng, :, :].rearrange("p r two w -> p (r two w)")
                            if s == "P":
                                nc.scalar.copy(
                                    pfb[:, 2 * g0 : 2 * g0 + 2 * ng, :].rearrange("p r w -> p (r w)"),
                                    ps_flat,
                                )
                            else:
                                d = p4.tile([128, 8, W2], F32, tag="d4")
                                d_flat = d[:, 0 : 2 * ng, :].rearrange("p r w -> p (r w)")
                                nc.vector.scalar_tensor_tensor(
                                    d_flat,
                                    ps_flat,
                                    0.0,
                                    pfb[:, 2 * g0 : 2 * g0 + 2 * ng, :].rearrange("p r w -> p (r w)"),
                                    OP.add,
                                    OP.subtract,
                                )
                                d2 = p4.tile([128, 8, W2], F32, tag="d42")
                                slot = SLOT_PERC + 2 * bi + g0 // 4
                                nc.scalar.activation(
                                    d2[:, 0 : 2 * ng, :].rearrange("p r w -> p (r w)"),
                                    d_flat,
                                    ACT.Square,
                                    accum_out=acc[:, slot : slot + 1],
                                )
                        if s == "P":
                            nc.sync.dma_start(pf[:, a - 3 : a - 3 + B, :], pfb[:])

            # =========== MSE ===========
            with tc.tile_pool(name="mse", bufs=2) as pmse:
                for ti, r0 in enumerate((6, 134)):
                    pt = pmse.tile([128, 2, W], F32, tag="msep")
                    nc.sync.dma_start(
                        pt[:], pred[:, r0 : r0 + 128, :].rearrange("c r w -> r c w")
                    )
                    tt = pmse.tile([128, 2, W], F32, tag="mset")
                    nc.sync.dma_start(
                        tt[:], targ[:, r0 : r0 + 128, :].rearrange("c r w -> r c w")
                    )
                    d = pmse.tile([128, 2, W], F32, tag="msed")
                    nc.vector.scalar_tensor_tensor(d[:], pt[:], 0.0, tt[:], OP.add, OP.subtract)
                    d2 = pmse.tile([128, 2, W], F32, tag="msed2")
                    nc.scalar.activation(
                        d2[:].rearrange("p c w -> p (c w)"),
                        d[:].rearrange("p c w -> p (c w)"),
                        ACT.Square,
                        accum_out=acc[:, SLOT_MSE + ti : SLOT_MSE + ti + 1],
                    )

            # =========== SSIM ===========
            with (
                tc.tile_pool(name="ssim", bufs=1) as ps_,
                tc.tile_pool(name="ssimw", bufs=2) as pw_,
                tc.tile_pool(name="pssim", bufs=5, space="PSUM") as pps,
            ):
                for ch in range(2):
                    tiles = {}
                    tdefs = ((0, 1, 128), (1, 129, 128), (2, 257, 10))
                    for tid, tr0, tnr in tdefs:
                        mp_ = {}
                        mp_["p"] = ps_.tile([tnr, W], F32, tag=f"sp{tid}", name=f"sp{tid}")
                        nc.sync.dma_start(mp_["p"][:], pred[ch, tr0 : tr0 + tnr, :])
                        mp_["t"] = ps_.tile([tnr, W], F32, tag=f"st{tid}", name=f"st{tid}")
                        nc.sync.dma_start(mp_["t"][:], targ[ch, tr0 : tr0 + tnr, :])
                        mp_["pp"] = ps_.tile([tnr, W], F32, tag=f"spp{tid}", name=f"spp{tid}")
                        nc.vector.tensor_mul(mp_["pp"][:], mp_["p"][:], mp_["p"][:])
                        mp_["tt"] = ps_.tile([tnr, W], F32, tag=f"stt{tid}", name=f"stt{tid}")
                        nc.vector.tensor_mul(mp_["tt"][:], mp_["t"][:], mp_["t"][:])
                        mp_["pt"] = ps_.tile([tnr, W], F32, tag=f"spt{tid}", name=f"spt{tid}")
                        nc.vector.tensor_mul(mp_["pt"][:], mp_["p"][:], mp_["t"][:])
                        tiles[tid] = mp_
                    for oi in range(2):
                        # O0 <- bandA@T0 + bandB@T1 ; O1 <- bandA@T1 + bandB[0:10]@T2
                        srcs = ((0, bAs[:], 128), (1, bBs[:], 128)) if oi == 0 else (
                            (1, bAs[:], 128), (2, bBs[0:10, :], 10))
                        vs = {}
                        for mname in ("p", "t", "pp", "tt", "pt"):
                            psv = pps.tile([128, W], F32, tag="psv")
                            for si_, (tid, band_ap, kk) in enumerate(srcs):
                                nc.tensor.matmul(
                                    psv[:],
                                    band_ap,
                                    tiles[tid][mname][:],
                                    start=(si_ == 0),
                                    stop=(si_ == 1),
                                )
                            wb = pw_.tile([128, 522], F32, tag=f"wb{mname}")
                            pad = bass.AP(wb[:].tensor, wb[:].offset, [[522, 128], [517, 2], [1, 5]])
                            nc.vector.memset(pad, 0.0)
                            nc.scalar.copy(wb[:, 5:517], psv[:])
                            # horizontal 11-tap sliding sum via log-shifts
                            s2 = pw_.tile([128, 522], F32, tag="s2")
                            nc.vector.tensor_add(s2[:, 0:521], wb[:, 0:521], wb[:, 1:522])
                            s3 = pw_.tile([128, 522], F32, tag="s3")
                            nc.vector.tensor_add(s3[:, 0:520], s2[:, 0:520], wb[:, 2:522])
                            s4 = pw_.tile([128, 522], F32, tag="s4")
                            nc.vector.tensor_add(s4[:, 0:517], s2[:, 0:517], s2[:, 2:519])
                            s8 = pw_.tile([128, 522], F32, tag="s8")
                            nc.vector.tensor_add(s8[:, 0:513], s4[:, 0:513], s4[:, 4:517])
                            sv = pw_.tile([128, W], F32, tag=f"sv{mname}")
                            nc.vector.tensor_add(sv[:], s8[:, 0:512], s3[:, 8:520])
                            vs[mname] = sv
                        # SSIM formula from window sums (mu = s/121)
                        A = pw_.tile([128, W], F32, tag="fA")
                        nc.vector.tensor_mul(A[:], vs["p"][:], vs["t"][:])
                        num1 = pw_.tile([128, W], F32, tag="fnum1")
                        nc.vector.tensor_scalar(num1[:], A[:], 2.0 * INV121SQ, C1, OP.mult, OP.add)
                        t1 = pw_.tile([128, W], F32, tag="ft1")
                        nc.vector.tensor_scalar(t1[:], vs["pt"][:], 2.0 * INV121, C2, OP.mult, OP.add)
                        num2 = pw_.tile([128, W], F32, tag="fnum2")
                        nc.vector.scalar_tensor_tensor(num2[:], A[:], -2.0 * INV121SQ, t1[:], OP.mult, OP.add)
                        sq1 = pw_.tile([128, W], F32, tag="fsq1")
                        nc.vector.tensor_mul(sq1[:], vs["p"][:], vs["p"][:])
                        sq2 = pw_.tile([128, W], F32, tag="fsq2")
                        nc.vector.tensor_mul(sq2[:], vs["t"][:], vs["t"][:])
                        ssum = pw_.tile([128, W], F32, tag="fssum")
                        nc.vector.tensor_add(ssum[:], sq1[:], sq2[:])
                        den1 = pw_.tile([128, W], F32, tag="fden1")
                        nc.vector.tensor_scalar(den1[:], ssum[:], INV121SQ, C1, OP.mult, OP.add)
                        u = pw_.tile([128, W], F32, tag="fu")
                        nc.vector.tensor_add(u[:], vs["pp"][:], vs["tt"][:])
                        u2 = pw_.tile([128, W], F32, tag="fu2")
                        nc.vector.tensor_scalar(u2[:], u[:], INV121, C2, OP.mult, OP.add)
                        den2 = pw_.tile([128, W], F32, tag="fden2")
                        nc.vector.scalar_tensor_tensor(den2[:], ssum[:], -INV121SQ, u2[:], OP.mult, OP.add)
                        num = pw_.tile([128, W], F32, tag="fnum")
                        nc.vector.tensor_mul(num[:], num1[:], num2[:])
                        den = pw_.tile([128, W], F32, tag="fden")
                        nc.vector.tensor_mul(den[:], den1[:], den2[:])
                        rden = pw_.tile([128, W], F32, tag="frden")
                        nc.vector.reciprocal(rden[:], den[:])
                        smap = pw_.tile([128, W], F32, tag="fsmap")
                        slot = SLOT_SSIM + 2 * ch + oi
                        nc.vector.scalar_tensor_tensor(
                            smap[:], num[:], 0.0, rden[:], OP.add, OP.mult,
                            accum_out=acc[:, slot : slot + 1],
                        )

            nc.sync.dma_start(parts_out[:], acc[:])

    _legalize_waits(nc)
    return nc, ins


_CACHE = {}


def _get_program():
    if "nc" not in _CACHE:
        _CACHE["nc"] = build_program()
    return _CACHE["nc"]


def _host_inputs(pred, target, w1, b1, w2, b2, w3, b3, w4, b4):
    """Build the 8 per-core input maps."""
    w1i = np.ascontiguousarray(np.transpose(w1, (1, 2, 3, 0)).reshape(27, 64))
    w2t = np.ascontiguousarray(np.transpose(w2, (2, 3, 1, 0)).reshape(9, 64, 64))
    w3t = np.ascontiguousarray(np.transpose(w3, (2, 3, 1, 0)).reshape(9, 64, 128))
    w4t = np.ascontiguousarray(np.transpose(w4, (2, 3, 1, 0)).reshape(9, 128, 128))
    b1d = np.concatenate([b1, b1]).reshape(128, 1).astype(np.float32)
    b2d = np.concatenate([b2, b2]).reshape(128, 1).astype(np.float32)
    b3d = b3.reshape(128, 1).astype(np.float32)
    k = np.arange(128)[:, None]
    m = np.arange(128)[None, :]
    bandA = ((k >= m) & (k < m + 11)).astype(np.float32)
    bandB = ((k >= m - 128) & (k < m - 117)).astype(np.float32)

    in_maps = []
    for c in range(8):
        b, h = divmod(c, 2)
        g0 = 256 * h - 6
        maps = {}
        for nm, full in (("pred", pred), ("targ", target)):
            slab = np.zeros((2, RSLAB, W), np.float32)
            lo, hi = max(0, g0), min(512, g0 + RSLAB)
            slab[:, lo - g0 : hi - g0, :] = full[b][:, lo:hi, :]
            maps[nm] = slab
        maps.update(
            w1i=w1i, w2t=w2t, w3t=w3t, w4t=w4t,
            b1d=b1d, b2d=b2d, b3d=b3d, bandA=bandA, bandB=bandB,
        )
        in_maps.append(maps)
    return in_maps


def _combine(results):
    mse_s = 0.0
    ssim_s = 0.0
    perc_s = 0.0
    for r in results:
        p = r["parts"].astype(np.float64)
        mse_s += p[:, SLOT_MSE : SLOT_MSE + 2].sum()
        ssim_s += p[:, SLOT_SSIM : SLOT_SSIM + 4].sum()
        perc_s += p[:, SLOT_PERC : SLOT_PERC + 16].sum()
    n_px = 4 * 2 * 512 * 512
    n_pc = 4 * 128 * 256 * 256
    loss = (
        mse_s / n_px
        + 0.2 * (perc_s / n_pc)
        + 0.2 * (1.0 - ssim_s / n_px)
    )
    return np.float32(loss)


def kernel(pred, target, w1, b1, w2, b2, w3, b3, w4, b4, _trace=False):
    nc, _ = _get_program()
    in_maps = _host_inputs(pred, target, w1, b1, w2, b2, w3, b3, w4, b4)
    res = run_bass_kernel_spmd(nc, in_maps, core_ids=list(range(8)), trace=_trace)
    out = _combine(res.results)
    if _trace:
        return out, res
    return out


# revision 14
# speedup vs baseline: 1.2162x; 1.2162x over previous
"""Trainium2 Bass kernel for ColorizationLoss (MSE + 0.2*VGG-perceptual + 0.2*SSIM).

Sharding: 8 cores; core c handles batch b=c//2, row-half h=c%2 (rows
[256h, 256h+256) of the 512-row image) for BOTH pred and target streams.
Each core gets a 268-row slab (halo 6 rows each side, zero-padded at true
image edges, matching SAME conv / SSIM zero padding). Per-core partial
sums are returned as [128, 24] per-partition accumulators; the host
combines them into the scalar loss.
"""
import sys

sys.path.insert(0, "/opt/trn_rl_repo")

import numpy as np
import concourse.bass as bass
import concourse.tile as tile
from concourse import mybir
from concourse.bass_utils import run_bass_kernel_spmd
from concourse.vector_clock import ScopedClock

F32 = mybir.dt.float32
F32R = mybir.dt.float32r
OP = mybir.AluOpType
ACT = mybir.ActivationFunctionType

# ---------------------------------------------------------------------------
# Walrus in this container caps sync waits at 1 per non-EventSemaphore
# instruction; TileContext's exit drain piles every global-clock wait onto a
# single Drain. Spill the extras onto standalone NOPs.
def _patched_drain_and_barrier(self, tick_clock, wait_clock):
    nc = self.nc
    drain_inst = nc.sync.drain()
    wait_clock.add_sem_waits(
        drain_inst.ins, ScopedClock({None: tick_clock.global_clock})
    )
    si = drain_inst.ins.sync_info
    if si is not None and len(si.on_wait) > 1:
        extras = list(si.on_wait[1:])
        drain_inst.ins.sync_info = mybir.SyncInfo(
            on_wait=[si.on_wait[0]], on_update=list(si.on_update)
        )
        for w in extras:
            nop = nc.sync.add_instruction(
                mybir.InstNoOp(name=nc.get_next_instruction_name(), ins=[], outs=[])
            )
            nop.ins.sync_info = mybir.SyncInfo(on_wait=[w], on_update=[])
    nc.all_engine_barrier()
    popped = nc._tile_sem_poison_stack.pop()
    assert popped is self._sem_poison
    nc.clear_and_free_semaphores(list(self.sems.allocated().values()))
    nc.all_engine_barrier()


tile.TileContext._drain_and_barrier = _patched_drain_and_barrier


def _legalize_waits(nc):
    """Split multi-wait instructions: this walrus caps sync waits at 1 per
    instruction (2 for EventSemaphore). Hoist extras onto same-engine NOPs
    inserted immediately before the instruction."""
    for f in nc.m.functions:
        for bb in f.blocks:
            new = []
            for inst in bb.instructions:
                cap = 2 if isinstance(inst, mybir.InstEventSemaphore) else 1
                si = inst.sync_info
                if si is not None and len(si.on_wait) > cap:
                    extras = list(si.on_wait[: len(si.on_wait) - cap])
                    keep = list(si.on_wait[len(si.on_wait) - cap :])
                    inst.sync_info = mybir.SyncInfo(
                        on_wait=keep, on_update=list(si.on_update)
                    )
                    for w in extras:
                        nop = mybir.InstNoOp(
                            name=nc.get_next_instruction_name(), ins=[], outs=[]
                        )
                        nop.engine = inst.engine
                        nop.sync_info = mybir.SyncInfo(on_wait=[w], on_update=[])
                        new.append(nop)
                new.append(inst)
            bb.instructions[:] = new

# ---------------------------------------------------------------------------
# Geometry (slab coords: slab row r == global row 256*h - 6 + r)
RSLAB = 268          # slab rows at 512 res
W = 512
WP = W + 2           # padded width at 512 res
W2 = 256             # 256-res width
W2P = W2 + 2

# fixed constants of the loss
C1 = 0.01 ** 2
C2 = 0.03 ** 2
INV121 = 1.0 / 121.0
INV121SQ = 1.0 / (121.0 * 121.0)
DELTA = 6.0 / 29.0
FY0 = 16.0 / 116.0
LIN_A = 3.0 * DELTA * DELTA      # finv linear slope
LIN_B = 4.0 / 29.0               # finv linear offset
# rgb = M @ [X, Z] with Y == 0 (L == 0 everywhere)
CXR = 3.240479 * 0.950456
CZR = -0.498535 * 1.088754
CXG = -0.969256 * 0.950456
CZG = 0.041556 * 1.088754
CXB = 0.055648 * 0.950456
CZB = 1.057311 * 1.088754

N_SLOTS = 24
SLOT_MSE = 0         # 2 slots
SLOT_SSIM = 2        # 4 slots
SLOT_PERC = 6        # 16 slots


def _bands(start, end, bh):
    out = []
    a = start
    while a < end:
        out.append((a, min(bh, end - a)))
        a += bh
    return out


def build_program():
    nc = bass.Bass()

    pred = nc.declare_dram_parameter("pred", [2, RSLAB, W], F32, isOutput=False)
    targ = nc.declare_dram_parameter("targ", [2, RSLAB, W], F32, isOutput=False)
    w1i = nc.declare_dram_parameter("w1i", [27, 64], F32R, isOutput=False)
    w2p = nc.declare_dram_parameter("w2p", [3, 128, 64], F32R, isOutput=False)
    w2r = nc.declare_dram_parameter("w2r", [3, 64, 64], F32R, isOutput=False)
    w3p = nc.declare_dram_parameter("w3p", [3, 128, 128], F32R, isOutput=False)
    w3r = nc.declare_dram_parameter("w3r", [3, 64, 128], F32R, isOutput=False)
    w4t = nc.declare_dram_parameter("w4t", [9, 128, 128], F32R, isOutput=False)
    b1d = nc.declare_dram_parameter("b1d", [128, 1], F32, isOutput=False)
    b2d = nc.declare_dram_parameter("b2d", [128, 1], F32, isOutput=False)
    b3d = nc.declare_dram_parameter("b3d", [128, 1], F32, isOutput=False)
    bandA = nc.declare_dram_parameter("bandA", [128, 128], F32, isOutput=False)
    bandB = nc.declare_dram_parameter("bandB", [128, 128], F32, isOutput=False)
    parts_out = nc.declare_dram_parameter("parts", [128, N_SLOTS], F32, isOutput=True)

    # per-stream HBM scratch
    scr = {}
    for s in ("P", "T"):
        scr[s] = dict(
            rgb=nc.dram_tensor(f"rgb{s}", [3, RSLAB, WP], F32R),
            c1=nc.dram_tensor(f"c1{s}", [64, RSLAB, WP], F32R),
            pl=nc.dram_tensor(f"pl{s}", [64, 134, W2P], F32R),
            c3=nc.dram_tensor(f"c3{s}", [128, 134, W2P], F32R),
        )
    pf = nc.dram_tensor("pf", [128, 128, W2], F32)

    ins = {"pred": pred, "targ": targ}

    with tile.TileContext(nc) as tc:
        with tc.tile_pool(name="const", bufs=1) as cpool:
            # constants in SBUF
            w1s = cpool.tile([27, 64], F32R)
            nc.sync.dma_start(w1s[:], w1i[:])
            w2ps = cpool.tile([128, 3, 64], F32R)
            nc.sync.dma_start(w2ps[:], w2p[:].rearrange("t c o -> c t o"))
            w2rs = cpool.tile([64, 3, 64], F32R)
            nc.sync.dma_start(w2rs[:], w2r[:].rearrange("t c o -> c t o"))
            w3ps = cpool.tile([128, 3, 128], F32R)
            nc.sync.dma_start(w3ps[:], w3p[:].rearrange("t c o -> c t o"))
            w3rs = cpool.tile([64, 3, 128], F32R)
            nc.sync.dma_start(w3rs[:], w3r[:].rearrange("t c o -> c t o"))
            w4s = cpool.tile([128, 9, 128], F32R)
            nc.sync.dma_start(w4s[:], w4t[:].rearrange("t c o -> c t o"))
            b1s = cpool.tile([128, 1], F32)
            nc.sync.dma_start(b1s[:], b1d[:])
            b2s = cpool.tile([128, 1], F32)
            nc.sync.dma_start(b2s[:], b2d[:])
            b3s = cpool.tile([128, 1], F32)
            nc.sync.dma_start(b3s[:], b3d[:])
            bAs = cpool.tile([128, 128], F32)
            nc.sync.dma_start(bAs[:], bandA[:])
            bBs = cpool.tile([128, 128], F32)
            nc.sync.dma_start(bBs[:], bandB[:])
            zt = cpool.tile([128, 536], F32R)
            nc.vector.memset(zt[:].bitcast(F32), 0.0)
            acc = cpool.tile([128, N_SLOTS], F32)
            nc.vector.memset(acc[:], 0.0)

            # zero the padding columns of the padded scratch tensors
            for s in ("P", "T"):
                for t_, ch, nr, wp in (
                    (scr[s]["rgb"], 3, RSLAB, WP),
                    (scr[s]["c1"], 64, RSLAB, WP),
                    (scr[s]["pl"], 64, 134, W2P),
                    (scr[s]["c3"], 128, 134, W2P),
                ):
                    for col in (0, wp - 1):
                        nc.sync.dma_start(t_[:, :, col : col + 1], zt[0:ch, 0:nr])

            # =========== per-stream VGG pipeline ===========
            for s, src in (("P", pred), ("T", targ)):
                sc = scr[s]
                # ---- LAB -> RGB (pixel-major tiles) ----
                with tc.tile_pool(name="lab", bufs=2) as lp:
                    for tr0, tnr in ((0, 128), (128, 128), (256, 12)):
                        ab = lp.tile([tnr, 2, W], F32, tag="ab")
                        nc.sync.dma_start(
                            ab[:], src[:, tr0 : tr0 + tnr, :].rearrange("c r w -> r c w")
                        )
                        a_ = ab[:, 0, :]
                        b_ = ab[:, 1, :]
                        fv = {}
                        for name, chan, c0 in (("x", a_, 128.0 / 500.0), ("z", b_, -128.0 / 200.0)):
                            f = lp.tile([tnr, W], F32, tag=f"f{name}")
                            nc.vector.tensor_scalar(f[:], chan, c0, FY0, OP.mult, OP.add)
                            cube = lp.tile([tnr, W], F32, tag=f"cube{name}")
                            nc.vector.tensor_mul(cube[:], f[:], f[:])
                            nc.vector.tensor_mul(cube[:], cube[:], f[:])
                            lin = lp.tile([tnr, W], F32, tag=f"lin{name}")
                            nc.vector.tensor_scalar(lin[:], f[:], LIN_B, LIN_A, OP.subtract, OP.mult)
                            mask = lp.tile([tnr, W], mybir.dt.int8, tag=f"mask{name}")
                            nc.vector.tensor_single_scalar(mask[:], f[:], DELTA, OP.is_gt)
                            v = lp.tile([tnr, W], F32, tag=f"v{name}")
                            nc.vector.select(v[:], mask[:], cube[:], lin[:])
                            fv[name] = v
                        rgb = lp.tile([tnr, 3, W], F32, tag="rgb")
                        for ci, (cx, cz) in enumerate(((CXR, CZR), (CXG, CZG), (CXB, CZB))):
                            tmp = lp.tile([tnr, W], F32, tag="ztmp")
                            nc.vector.tensor_scalar_mul(tmp[:], fv["z"][:], cz)
                            chn = lp.tile([tnr, W], F32, tag="chn")
                            nc.vector.scalar_tensor_tensor(chn[:], fv["x"][:], cx, tmp[:], OP.mult, OP.add)
                            # clip to [1e-8, 1]
                            nc.vector.tensor_scalar(chn[:], chn[:], 1e-8, 1.0, OP.max, OP.min)
                            maskg = lp.tile([tnr, W], mybir.dt.int8, tag="maskg")
                            nc.vector.tensor_single_scalar(maskg[:], chn[:], 0.0031308, OP.is_le)
                            ling = lp.tile([tnr, W], F32, tag="ling")
                            nc.vector.tensor_scalar_mul(ling[:], chn[:], 12.92)
                            lnv = lp.tile([tnr, W], F32, tag="lnv")
                            nc.scalar.activation(lnv[:], chn[:], ACT.Ln)
                            pw = lp.tile([tnr, W], F32, tag="pw")
                            nc.scalar.activation(pw[:], lnv[:], ACT.Exp, scale=1.0 / 2.4)
                            nc.vector.tensor_scalar(pw[:], pw[:], 1.055, 0.055, OP.mult, OP.subtract)
                            nc.vector.select(rgb[:, ci, :], maskg[:], ling[:], pw[:])
                        nc.sync.dma_start(
                            sc["rgb"][:, tr0 : tr0 + tnr, 1 : 1 + W].rearrange("c r w -> r c w").bitcast(F32),
                            rgb[:],
                        )

                # ---- conv1: K=27 im2col, 1 matmul per output row ----
                with (
                    tc.tile_pool(name="cv1", bufs=2) as p1,
                    tc.tile_pool(name="ps1", bufs=2, space="PSUM") as pp1,
                ):
                    rgbt = sc["rgb"]
                    for a, B in _bands(1, 267, 16):
                        im = p1.tile([27, B, W], F32R, tag="im")
                        for c in range(3):
                            for dy in range(3):
                                src_ap = bass.AP(
                                    rgbt[:].tensor,
                                    c * (RSLAB * WP) + (a - 1 + dy) * WP,
                                    [[1, 3], [WP, B], [1, W]],
                                )
                                p0 = 9 * c + 3 * dy
                                nc.sync.dma_start(im[p0 : p0 + 3, :, :], src_ap)
                        obuf = p1.tile([64, B, W], F32R, tag="obuf")
                        for j0 in range(0, B, 4):
                            g = min(4, B - j0)
                            ps = pp1.tile([64, 4, W], F32, tag="ps")
                            for j in range(j0, j0 + g):
                                nc.tensor.matmul(
                                    ps[:, j - j0, :],
                                    w1s[:],
                                    im[:, j, :],
                                    start=True,
                                    stop=True,
                                )
                            nc.scalar.activation(
                                obuf[:, j0 : j0 + g, :].rearrange("p r w -> p (r w)"),
                                ps[:, 0:g, :].rearrange("p r w -> p (r w)"),
                                ACT.Relu,
                                bias=b1s[0:64, :],
                            )
                        nc.sync.dma_start(sc["c1"][:, a : a + B, 1 : 1 + W], obuf[:])

                # ---- conv2: dy-pair packed K=128 (+ dy=2 K=64), fused maxpool ----
                with (
                    tc.tile_pool(name="cv2", bufs=2) as p2,
                    tc.tile_pool(name="ps2", bufs=2, space="PSUM") as pp2,
                ):
                    for a, B in _bands(2, 266, 16):
                        # parts 0:64 = c1 rows [a-1, a+B+1); parts 64:128 = rows [a, a+B+2)
                        inb = p2.tile([128, B + 2, WP], F32R, tag="inb")
                        nc.sync.dma_start(inb[0:64, :, :], sc["c1"][:, a - 1 : a + B + 1, :])
                        nc.sync.dma_start(inb[64:128, :, :], sc["c1"][:, a : a + B + 2, :])
                        obuf = p2.tile([64, B, W], F32, tag="obuf2")
                        for j0 in range(0, B, 4):
                            g = min(4, B - j0)
                            ps = pp2.tile([64, 4, W], F32, tag="ps2")
                            for j in range(j0, j0 + g):
                                for dx in range(3):
                                    nc.tensor.matmul(
                                        ps[:, j - j0, :],
                                        w2ps[:, dx, :],
                                        inb[:, j, dx : dx + W],
                                        start=(dx == 0),
                                        stop=False,
                                    )
                                for dx in range(3):
                                    nc.tensor.matmul(
                                        ps[:, j - j0, :],
                                        w2rs[:, dx, :],
                                        inb[0:64, j + 2, dx : dx + W],
                                        start=False,
                                        stop=(dx == 2),
                                    )
                            nc.scalar.activation(
                                obuf[:, j0 : j0 + g, :].rearrange("p r w -> p (r w)"),
                                ps[:, 0:g, :].rearrange("p r w -> p (r w)"),
                                ACT.Relu,
                                bias=b2s[0:64, :],
                            )
                        # fused maxpool 2x2 on this band (a is even, B even)
                        rview = obuf[:].rearrange("c (r two) w -> c r two w", two=2)
                        tmp = p2.tile([64, B // 2, W], F32, tag="mptmp")
                        nc.vector.tensor_max(tmp[:], rview[:, :, 0, :], rview[:, :, 1, :])
                        cview = tmp[:].rearrange("c r (x two) -> c r x two", two=2)
                        ob = p2.tile([64, B // 2, W2], F32R, tag="mpout")
                        nc.vector.tensor_max(ob[:], cview[:, :, :, 0], cview[:, :, :, 1])
                        nc.sync.dma_start(
                            sc["pl"][:, a // 2 : a // 2 + B // 2, 1 : 1 + W2], ob[:]
                        )

                # ---- conv3: K=64 -> 128, 2 rows per matmul ----
                with (
                    tc.tile_pool(name="cv3", bufs=2) as p3,
                    tc.tile_pool(name="ps3", bufs=2, space="PSUM") as pp3,
                ):
                    for a, B in _bands(2, 132, 16):
                        # parts 0:64 = pl rows [a-1, a+B+1); parts 64:128 = rows [a, a+B+2)
                        inb = p3.tile([128, B + 2, W2P], F32R, tag="inb3")
                        nc.sync.dma_start(inb[0:64, :, :], sc["pl"][:, a - 1 : a + B + 1, :])
                        nc.sync.dma_start(inb[64:128, :, :], sc["pl"][:, a : a + B + 2, :])
                        obuf = p3.tile([128, B, W2], F32R, tag="obuf3")
                        npair = B // 2
                        for g0 in range(0, npair, 4):
                            ng = min(4, npair - g0)
                            ps = pp3.tile([128, 4, 2, W2], F32, tag="ps3")
                            for g in range(g0, g0 + ng):
                                for dx in range(3):
                                    nc.tensor.matmul(
                                        ps[:, g - g0, :, :],
                                        w3ps[:, dx, :],
                                        inb[:, 2 * g : 2 * g + 2, dx : dx + W2],
                                        start=(dx == 0),
                                        stop=False,
                                    )
                                for dx in range(3):
                                    nc.tensor.matmul(
                                        ps[:, g - g0, :, :],
                                        w3rs[:, dx, :],
                                        inb[0:64, 2 * g + 2 : 2 * g + 4, dx : dx + W2],
                                        start=False,
                                        stop=(dx == 2),
                                    )
                            nc.scalar.activation(
                                obuf[:, 2 * g0 : 2 * g0 + 2 * ng, :].rearrange("p r w -> p (r w)"),
                                ps[:, 0:ng, :, :].rearrange("p r two w -> p (r two w)"),
                                ACT.Relu,
                                bias=b3s[:],
                            )
                        nc.sync.dma_start(sc["c3"][:, a : a + B, 1 : 1 + W2], obuf[:])

                # ---- conv4 (+ store pf / diff-square-accumulate) ----
                with (
                    tc.tile_pool(name="cv4", bufs=2) as p4,
                    tc.tile_pool(name="ps4", bufs=2, space="PSUM") as pp4,
                ):
                    for bi, (a, B) in enumerate(_bands(3, 131, 16)):
                        inb = p4.tile([128, B + 2, W2P], F32R, tag="inb4")
                        nc.sync.dma_start(inb[:], sc["c3"][:, a - 1 : a + B + 1, :])
                        if s == "P":
                            pfb = p4.tile([128, B, W2], F32, tag="pfb")
                        else:
                            pfb = p4.tile([128, B, W2], F32, tag="pfb")
                            nc.sync.dma_start(pfb[:], pf[:, a - 3 : a - 3 + B, :])
                        npair = B // 2
                        for g0 in range(0, npair, 4):
                            ng = min(4, npair - g0)
                            ps = pp4.tile([128, 4, 2, W2], F32, tag="ps4")
                            for g in range(g0, g0 + ng):
                                for t9 in range(9):
                                    dy, dx = divmod(t9, 3)
                                    nc.tensor.matmul(
                                        ps[:, g - g0, :, :],
                                        w4s[:, t9, :],
                                        inb[:, 2 * g + dy : 2 * g + dy + 2, dx : dx + W2],
                                        start=(t9 == 0),
                                        stop=(t9 == 8),
                                    )
                            ps_flat = ps[:, 0:ng, :, :].rearrange("p r two w -> p (r two w)")
                            if s == "P":
                                nc.scalar.copy(
                                    pfb[:, 2 * g0 : 2 * g0 + 2 * ng, :].rearrange("p r w -> p (r w)"),
                                    ps_flat,
                                )
                            else:
                                d = p4.tile([128, 8, W2], F32, tag="d4")
                                d_flat = d[:, 0 : 2 * ng, :].rearrange("p r w -> p (r w)")
                                nc.vector.scalar_tensor_tensor(
                                    d_flat,
                                    ps_flat,
                                    0.0,
                                    pfb[:, 2 * g0 : 2 * g0 + 2 * ng, :].rearrange("p r w -> p (r w)"),
                                    OP.add,
                                    OP.subtract,
                                )
                                d2 = p4.tile([128, 8, W2], F32, tag="d42")
                                slot = SLOT_PERC + 2 * bi + g0 // 4
                                nc.scalar.activation(
                                    d2[:, 0 : 2 * ng, :].rearrange("p r w -> p (r w)"),
                                    d_flat,
                                    ACT.Square,
                                    accum_out=acc[:, slot : slot + 1],
                                )
                        if s == "P":
                            nc.sync.dma_start(pf[:, a - 3 : a - 3 + B, :], pfb[:])

            # =========== MSE ===========
            with tc.tile_pool(name="mse", bufs=2) as pmse:
                for ti, r0 in enumerate((6, 134)):
                    pt = pmse.tile([128, 2, W], F32, tag="msep")
                    nc.sync.dma_start(
                        pt[:], pred[:, r0 : r0 + 128, :].rearrange("c r w -> r c w")
                    )
                    tt = pmse.tile([128, 2, W], F32, tag="mset")
                    nc.sync.dma_start(
                        tt[:], targ[:, r0 : r0 + 128, :].rearrange("c r w -> r c w")
                    )
                    d = pmse.tile([128, 2, W], F32, tag="msed")
                    nc.vector.scalar_tensor_tensor(d[:], pt[:], 0.0, tt[:], OP.add, OP.subtract)
                    d2 = pmse.tile([128, 2, W], F32, tag="msed2")
                    nc.scalar.activation(
                        d2[:].rearrange("p c w -> p (c w)"),
                        d[:].rearrange("p c w -> p (c w)"),
                        ACT.Square,
                        accum_out=acc[:, SLOT_MSE + ti : SLOT_MSE + ti + 1],
                    )

            # =========== SSIM ===========
            with (
                tc.tile_pool(name="ssim", bufs=1) as ps_,
                tc.tile_pool(name="ssimw", bufs=2) as pw_,
                tc.tile_pool(name="pssim", bufs=5, space="PSUM") as pps,
            ):
                for ch in range(2):
                    tiles = {}
                    tdefs = ((0, 1, 128), (1, 129, 128), (2, 257, 10))
                    for tid, tr0, tnr in tdefs:
                        mp_ = {}
                        mp_["p"] = ps_.tile([tnr, W], F32, tag=f"sp{tid}", name=f"sp{tid}")
                        nc.sync.dma_start(mp_["p"][:], pred[ch, tr0 : tr0 + tnr, :])
                        mp_["t"] = ps_.tile([tnr, W], F32, tag=f"st{tid}", name=f"st{tid}")
                        nc.sync.dma_start(mp_["t"][:], targ[ch, tr0 : tr0 + tnr, :])
                        mp_["pp"] = ps_.tile([tnr, W], F32, tag=f"spp{tid}", name=f"spp{tid}")
                        nc.vector.tensor_mul(mp_["pp"][:], mp_["p"][:], mp_["p"][:])
                        mp_["tt"] = ps_.tile([tnr, W], F32, tag=f"stt{tid}", name=f"stt{tid}")
                        nc.vector.tensor_mul(mp_["tt"][:], mp_["t"][:], mp_["t"][:])
                        mp_["pt"] = ps_.tile([tnr, W], F32, tag=f"spt{tid}", name=f"spt{tid}")
                        nc.vector.tensor_mul(mp_["pt"][:], mp_["p"][:], mp_["t"][:])
                        tiles[tid] = mp_
                    for oi in range(2):
                        # O0 <- bandA@T0 + bandB@T1 ; O1 <- bandA@T1 + bandB[0:10]@T2
                        srcs = ((0, bAs[:], 128), (1, bBs[:], 128)) if oi == 0 else (
                            (1, bAs[:], 128), (2, bBs[0:10, :], 10))
                        vs = {}
                        for mname in ("p", "t", "pp", "tt", "pt"):
                            psv = pps.tile([128, W], F32, tag="psv")
                            for si_, (tid, band_ap, kk) in enumerate(srcs):
                                nc.tensor.matmul(
                                    psv[:],
                                    band_ap,
                                    tiles[tid][mname][:],
                                    start=(si_ == 0),
                                    stop=(si_ == 1),
                                )
                            wb = pw_.tile([128, 522], F32, tag=f"wb{mname}")
                            pad = bass.AP(wb[:].tensor, wb[:].offset, [[522, 128], [517, 2], [1, 5]])
                            nc.vector.memset(pad, 0.0)
                            nc.scalar.copy(wb[:, 5:517], psv[:])
                            # horizontal 11-tap sliding sum via log-shifts
                            s2 = pw_.tile([128, 522], F32, tag="s2")
                            nc.vector.tensor_add(s2[:, 0:521], wb[:, 0:521], wb[:, 1:522])
                            s3 = pw_.tile([128, 522], F32, tag="s3")
                            nc.vector.tensor_add(s3[:, 0:520], s2[:, 0:520], wb[:, 2:522])
                            s4 = pw_.tile([128, 522], F32, tag="s4")
                            nc.vector.tensor_add(s4[:, 0:517], s2[:, 0:517], s2[:, 2:519])
                            s8 = pw_.tile([128, 522], F32, tag="s8")
                            nc.vector.tensor_add(s8[:, 0:513], s4[:, 0:513], s4[:, 4:517])
                            sv = pw_.tile([128, W], F32, tag=f"sv{mname}")
                            nc.vector.tensor_add(sv[:], s8[:, 0:512], s3[:, 8:520])
                            vs[mname] = sv
                        # SSIM formula from window sums (mu = s/121)
                        A = pw_.tile([128, W], F32, tag="fA")
                        nc.vector.tensor_mul(A[:], vs["p"][:], vs["t"][:])
                        num1 = pw_.tile([128, W], F32, tag="fnum1")
                        nc.vector.tensor_scalar(num1[:], A[:], 2.0 * INV121SQ, C1, OP.mult, OP.add)
                        t1 = pw_.tile([128, W], F32, tag="ft1")
                        nc.vector.tensor_scalar(t1[:], vs["pt"][:], 2.0 * INV121, C2, OP.mult, OP.add)
                        num2 = pw_.tile([128, W], F32, tag="fnum2")
                        nc.vector.scalar_tensor_tensor(num2[:], A[:], -2.0 * INV121SQ, t1[:], OP.mult, OP.add)
                        sq1 = pw_.tile([128, W], F32, tag="fsq1")
                        nc.vector.tensor_mul(sq1[:], vs["p"][:], vs["p"][:])
                        sq2 = pw_.tile([128, W], F32, tag="fsq2")
                        nc.vector.tensor_mul(sq2[:], vs["t"][:], vs["t"][:])
                        ssum = pw_.tile([128, W], F32, tag="fssum")
                        nc.vector.tensor_add(ssum[:], sq1[:], sq2[:])
                        den1 = pw_.tile([128, W], F32, tag="fden1")
                        nc.vector.tensor_scalar(den1[:], ssum[:], INV121SQ, C1, OP.mult, OP.add)
                        u = pw_.tile([128, W], F32, tag="fu")
                        nc.vector.tensor_add(u[:], vs["pp"][:], vs["tt"][:])
                        u2 = pw_.tile([128, W], F32, tag="fu2")
                        nc.vector.tensor_scalar(u2[:], u[:], INV121, C2, OP.mult, OP.add)
                        den2 = pw_.tile([128, W], F32, tag="fden2")
                        nc.vector.scalar_tensor_tensor(den2[:], ssum[:], -INV121SQ, u2[:], OP.mult, OP.add)
                        num = pw_.tile([128, W], F32, tag="fnum")
                        nc.vector.tensor_mul(num[:], num1[:], num2[:])
                        den = pw_.tile([128, W], F32, tag="fden")
                        nc.vector.tensor_mul(den[:], den1[:], den2[:])
                        rden = pw_.tile([128, W], F32, tag="frden")
                        nc.vector.reciprocal(rden[:], den[:])
                        smap = pw_.tile([128, W], F32, tag="fsmap")
                        slot = SLOT_SSIM + 2 * ch + oi
                        nc.vector.scalar_tensor_tensor(
                            smap[:], num[:], 0.0, rden[:], OP.add, OP.mult,
                            accum_out=acc[:, slot : slot + 1],
                        )

            nc.sync.dma_start(parts_out[:], acc[:])

    _legalize_waits(nc)
    return nc, ins


_CACHE = {}


def _get_program():
    if "nc" not in _CACHE:
        _CACHE["nc"] = build_program()
    return _CACHE["nc"]


def _host_inputs(pred, target, w1, b1, w2, b2, w3, b3, w4, b4):
    """Build the 8 per-core input maps."""
    w1i = np.ascontiguousarray(np.transpose(w1, (1, 2, 3, 0)).reshape(27, 64))
    wt2 = np.transpose(w2, (2, 3, 1, 0))  # [dy, dx, c, o]
    w2p_h = np.ascontiguousarray(
        np.stack([np.concatenate([wt2[0, dx], wt2[1, dx]], axis=0) for dx in range(3)])
    )
    w2r_h = np.ascontiguousarray(np.stack([wt2[2, dx] for dx in range(3)]))
    wt3 = np.transpose(w3, (2, 3, 1, 0))
    w3p_h = np.ascontiguousarray(
        np.stack([np.concatenate([wt3[0, dx], wt3[1, dx]], axis=0) for dx in range(3)])
    )
    w3r_h = np.ascontiguousarray(np.stack([wt3[2, dx] for dx in range(3)]))
    w4t = np.ascontiguousarray(np.transpose(w4, (2, 3, 1, 0)).reshape(9, 128, 128))
    b1d = np.concatenate([b1, b1]).reshape(128, 1).astype(np.float32)
    b2d = np.concatenate([b2, b2]).reshape(128, 1).astype(np.float32)
    b3d = b3.reshape(128, 1).astype(np.float32)
    k = np.arange(128)[:, None]
    m = np.arange(128)[None, :]
    bandA = ((k >= m) & (k < m + 11)).astype(np.float32)
    bandB = ((k >= m - 128) & (k < m - 117)).astype(np.float32)

    in_maps = []
    for c in range(8):
        b, h = divmod(c, 2)
        g0 = 256 * h - 6
        maps = {}
        for nm, full in (("pred", pred), ("targ", target)):
            slab = np.zeros((2, RSLAB, W), np.float32)
            lo, hi = max(0, g0), min(512, g0 + RSLAB)
            slab[:, lo - g0 : hi - g0, :] = full[b][:, lo:hi, :]
            maps[nm] = slab
        maps.update(
            w1i=w1i, w2p=w2p_h, w2r=w2r_h, w3p=w3p_h, w3r=w3r_h, w4t=w4t,
            b1d=b1d, b2d=b2d, b3d=b3d, bandA=bandA, bandB=bandB,
        )
        in_maps.append(maps)
    return in_maps


def _combine(results):
    mse_s = 0.0
    ssim_s = 0.0
    perc_s = 0.0
    for r in results:
        p = r["parts"].astype(np.float64)
        mse_s += p[:, SLOT_MSE : SLOT_MSE + 2].sum()
        ssim_s += p[:, SLOT_SSIM : SLOT_SSIM + 4].sum()
        perc_s += p[:, SLOT_PERC : SLOT_PERC + 16].sum()
    n_px = 4 * 2 * 512 * 512
    n_pc = 4 * 128 * 256 * 256
    loss = (
        mse_s / n_px
        + 0.2 * (perc_s / n_pc)
        + 0.2 * (1.0 - ssim_s / n_px)
    )
    return np.float32(loss)


def kernel(pred, target, w1, b1, w2, b2, w3, b3, w4, b4, _trace=False):
    nc, _ = _get_program()
    in_maps = _host_inputs(pred, target, w1, b1, w2, b2, w3, b3, w4, b4)
    res = run_bass_kernel_spmd(nc, in_maps, core_ids=list(range(8)), trace=_trace)
    out = _combine(res.results)
    if _trace:
        return out, res
    return out


# revision 15
# speedup vs baseline: 1.3298x; 1.0934x over previous
"""Trainium2 Bass kernel for ColorizationLoss (MSE + 0.2*VGG-perceptual + 0.2*SSIM).

Sharding: 8 cores; core c handles batch b=c//2, row-half h=c%2 (rows
[256h, 256h+256) of the 512-row image) for BOTH pred and target streams.
Each core gets a 268-row slab (halo 6 rows each side, zero-padded at true
image edges, matching SAME conv / SSIM zero padding). Per-core partial
sums are returned as [128, 24] per-partition accumulators; the host
combines them into the scalar loss.
"""
import sys

sys.path.insert(0, "/opt/trn_rl_repo")

import numpy as np
import ml_dtypes
import concourse.bass as bass
import concourse.tile as tile
from concourse import mybir
from concourse.bass_utils import run_bass_kernel_spmd
from concourse.vector_clock import ScopedClock

F32 = mybir.dt.float32
F32R = mybir.dt.float32r
OP = mybir.AluOpType
ACT = mybir.ActivationFunctionType
BF16 = mybir.dt.bfloat16

# ---------------------------------------------------------------------------
# Walrus in this container caps sync waits at 1 per non-EventSemaphore
# instruction; TileContext's exit drain piles every global-clock wait onto a
# single Drain. Spill the extras onto standalone NOPs.
def _patched_drain_and_barrier(self, tick_clock, wait_clock):
    nc = self.nc
    drain_inst = nc.sync.drain()
    wait_clock.add_sem_waits(
        drain_inst.ins, ScopedClock({None: tick_clock.global_clock})
    )
    si = drain_inst.ins.sync_info
    if si is not None and len(si.on_wait) > 1:
        extras = list(si.on_wait[1:])
        drain_inst.ins.sync_info = mybir.SyncInfo(
            on_wait=[si.on_wait[0]], on_update=list(si.on_update)
        )
        for w in extras:
            nop = nc.sync.add_instruction(
                mybir.InstNoOp(name=nc.get_next_instruction_name(), ins=[], outs=[])
            )
            nop.ins.sync_info = mybir.SyncInfo(on_wait=[w], on_update=[])
    nc.all_engine_barrier()
    popped = nc._tile_sem_poison_stack.pop()
    assert popped is self._sem_poison
    nc.clear_and_free_semaphores(list(self.sems.allocated().values()))
    nc.all_engine_barrier()


tile.TileContext._drain_and_barrier = _patched_drain_and_barrier


def _legalize_waits(nc):
    """Split multi-wait instructions: this walrus caps sync waits at 1 per
    instruction (2 for EventSemaphore). Hoist extras onto same-engine NOPs
    inserted immediately before the instruction."""
    for f in nc.m.functions:
        for bb in f.blocks:
            new = []
            for inst in bb.instructions:
                cap = 2 if isinstance(inst, mybir.InstEventSemaphore) else 1
                si = inst.sync_info
                if si is not None and len(si.on_wait) > cap:
                    extras = list(si.on_wait[: len(si.on_wait) - cap])
                    keep = list(si.on_wait[len(si.on_wait) - cap :])
                    inst.sync_info = mybir.SyncInfo(
                        on_wait=keep, on_update=list(si.on_update)
                    )
                    for w in extras:
                        nop = mybir.InstNoOp(
                            name=nc.get_next_instruction_name(), ins=[], outs=[]
                        )
                        nop.engine = inst.engine
                        nop.sync_info = mybir.SyncInfo(on_wait=[w], on_update=[])
                        new.append(nop)
                new.append(inst)
            bb.instructions[:] = new

# ---------------------------------------------------------------------------
# Geometry (slab coords: slab row r == global row 256*h - 6 + r)
RSLAB = 268          # slab rows at 512 res
W = 512
WP = W + 2           # padded width at 512 res
W2 = 256             # 256-res width
W2P = W2 + 2

# fixed constants of the loss
C1 = 0.01 ** 2
C2 = 0.03 ** 2
INV121 = 1.0 / 121.0
INV121SQ = 1.0 / (121.0 * 121.0)
DELTA = 6.0 / 29.0
FY0 = 16.0 / 116.0
LIN_A = 3.0 * DELTA * DELTA      # finv linear slope
LIN_B = 4.0 / 29.0               # finv linear offset
# rgb = M @ [X, Z] with Y == 0 (L == 0 everywhere)
CXR = 3.240479 * 0.950456
CZR = -0.498535 * 1.088754
CXG = -0.969256 * 0.950456
CZG = 0.041556 * 1.088754
CXB = 0.055648 * 0.950456
CZB = 1.057311 * 1.088754

N_SLOTS = 24
SLOT_MSE = 0         # 2 slots
SLOT_SSIM = 2        # 4 slots
SLOT_PERC = 6        # 16 slots


def _bands(start, end, bh):
    out = []
    a = start
    while a < end:
        out.append((a, min(bh, end - a)))
        a += bh
    return out


def build_program():
    nc = bass.Bass()

    pred = nc.declare_dram_parameter("pred", [2, RSLAB, W], F32, isOutput=False)
    targ = nc.declare_dram_parameter("targ", [2, RSLAB, W], F32, isOutput=False)
    w1i = nc.declare_dram_parameter("w1i", [27, 64], F32R, isOutput=False)
    w2p = nc.declare_dram_parameter("w2p", [3, 128, 64], BF16, isOutput=False)
    w2r = nc.declare_dram_parameter("w2r", [3, 64, 64], BF16, isOutput=False)
    w3p = nc.declare_dram_parameter("w3p", [3, 128, 128], BF16, isOutput=False)
    w3r = nc.declare_dram_parameter("w3r", [3, 64, 128], BF16, isOutput=False)
    w4t = nc.declare_dram_parameter("w4t", [9, 128, 128], BF16, isOutput=False)
    b1d = nc.declare_dram_parameter("b1d", [128, 1], F32, isOutput=False)
    b2d = nc.declare_dram_parameter("b2d", [128, 1], F32, isOutput=False)
    b3d = nc.declare_dram_parameter("b3d", [128, 1], F32, isOutput=False)
    bandA = nc.declare_dram_parameter("bandA", [128, 128], F32, isOutput=False)
    bandB = nc.declare_dram_parameter("bandB", [128, 128], F32, isOutput=False)
    parts_out = nc.declare_dram_parameter("parts", [128, N_SLOTS], F32, isOutput=True)

    # per-stream HBM scratch
    scr = {}
    for s in ("P", "T"):
        scr[s] = dict(
            rgb=nc.dram_tensor(f"rgb{s}", [3, RSLAB, WP], F32R),
            c1=nc.dram_tensor(f"c1{s}", [64, RSLAB, WP], BF16),
            pl=nc.dram_tensor(f"pl{s}", [64, 134, W2P], BF16),
            c3=nc.dram_tensor(f"c3{s}", [128, 134, W2P], BF16),
        )
    pf = nc.dram_tensor("pf", [128, 128, W2], F32)

    ins = {"pred": pred, "targ": targ}

    with tile.TileContext(nc) as tc:
        with tc.tile_pool(name="const", bufs=1) as cpool:
            # constants in SBUF
            w1s = cpool.tile([27, 64], F32R)
            nc.sync.dma_start(w1s[:], w1i[:])
            w2ps = cpool.tile([128, 3, 64], BF16)
            nc.sync.dma_start(w2ps[:], w2p[:].rearrange("t c o -> c t o"))
            w2rs = cpool.tile([64, 3, 64], BF16)
            nc.sync.dma_start(w2rs[:], w2r[:].rearrange("t c o -> c t o"))
            w3ps = cpool.tile([128, 3, 128], BF16)
            nc.sync.dma_start(w3ps[:], w3p[:].rearrange("t c o -> c t o"))
            w3rs = cpool.tile([64, 3, 128], BF16)
            nc.sync.dma_start(w3rs[:], w3r[:].rearrange("t c o -> c t o"))
            w4s = cpool.tile([128, 9, 128], BF16)
            nc.sync.dma_start(w4s[:], w4t[:].rearrange("t c o -> c t o"))
            b1s = cpool.tile([128, 1], F32)
            nc.sync.dma_start(b1s[:], b1d[:])
            b2s = cpool.tile([128, 1], F32)
            nc.sync.dma_start(b2s[:], b2d[:])
            b3s = cpool.tile([128, 1], F32)
            nc.sync.dma_start(b3s[:], b3d[:])
            bAs = cpool.tile([128, 128], F32)
            nc.sync.dma_start(bAs[:], bandA[:])
            bBs = cpool.tile([128, 128], F32)
            nc.sync.dma_start(bBs[:], bandB[:])
            zt = cpool.tile([128, 536], F32R)
            nc.vector.memset(zt[:].bitcast(F32), 0.0)
            ztb = cpool.tile([128, 536], BF16)
            nc.vector.memset(ztb[:], 0.0)
            acc = cpool.tile([128, N_SLOTS], F32)
            nc.vector.memset(acc[:], 0.0)

            # zero the padding columns of the padded scratch tensors
            for s in ("P", "T"):
                for t_, ch, nr, wp in (
                    (scr[s]["rgb"], 3, RSLAB, WP),
                    (scr[s]["c1"], 64, RSLAB, WP),
                    (scr[s]["pl"], 64, 134, W2P),
                    (scr[s]["c3"], 128, 134, W2P),
                ):
                    zsrc = zt if t_.dtype == F32R else ztb
                    for col in (0, wp - 1):
                        nc.sync.dma_start(t_[:, :, col : col + 1], zsrc[0:ch, 0:nr])

            # =========== per-stream VGG pipeline ===========
            for s, src in (("P", pred), ("T", targ)):
                sc = scr[s]
                # ---- LAB -> RGB (pixel-major tiles) ----
                with tc.tile_pool(name="lab", bufs=2) as lp:
                    for tr0, tnr in ((0, 128), (128, 128), (256, 12)):
                        ab = lp.tile([tnr, 2, W], F32, tag="ab")
                        nc.sync.dma_start(
                            ab[:], src[:, tr0 : tr0 + tnr, :].rearrange("c r w -> r c w")
                        )
                        a_ = ab[:, 0, :]
                        b_ = ab[:, 1, :]
                        fv = {}
                        for name, chan, c0 in (("x", a_, 128.0 / 500.0), ("z", b_, -128.0 / 200.0)):
                            f = lp.tile([tnr, W], F32, tag=f"f{name}")
                            nc.vector.tensor_scalar(f[:], chan, c0, FY0, OP.mult, OP.add)
                            cube = lp.tile([tnr, W], F32, tag=f"cube{name}")
                            nc.vector.tensor_mul(cube[:], f[:], f[:])
                            nc.vector.tensor_mul(cube[:], cube[:], f[:])
                            lin = lp.tile([tnr, W], F32, tag=f"lin{name}")
                            nc.vector.tensor_scalar(lin[:], f[:], LIN_B, LIN_A, OP.subtract, OP.mult)
                            mask = lp.tile([tnr, W], mybir.dt.int8, tag=f"mask{name}")
                            nc.vector.tensor_single_scalar(mask[:], f[:], DELTA, OP.is_gt)
                            v = lp.tile([tnr, W], F32, tag=f"v{name}")
                            nc.vector.select(v[:], mask[:], cube[:], lin[:])
                            fv[name] = v
                        rgb = lp.tile([tnr, 3, W], F32, tag="rgb")
                        for ci, (cx, cz) in enumerate(((CXR, CZR), (CXG, CZG), (CXB, CZB))):
                            tmp = lp.tile([tnr, W], F32, tag="ztmp")
                            nc.vector.tensor_scalar_mul(tmp[:], fv["z"][:], cz)
                            chn = lp.tile([tnr, W], F32, tag="chn")
                            nc.vector.scalar_tensor_tensor(chn[:], fv["x"][:], cx, tmp[:], OP.mult, OP.add)
                            # clip to [1e-8, 1]
                            nc.vector.tensor_scalar(chn[:], chn[:], 1e-8, 1.0, OP.max, OP.min)
                            maskg = lp.tile([tnr, W], mybir.dt.int8, tag="maskg")
                            nc.vector.tensor_single_scalar(maskg[:], chn[:], 0.0031308, OP.is_le)
                            ling = lp.tile([tnr, W], F32, tag="ling")
                            nc.vector.tensor_scalar_mul(ling[:], chn[:], 12.92)
                            lnv = lp.tile([tnr, W], F32, tag="lnv")
                            nc.scalar.activation(lnv[:], chn[:], ACT.Ln)
                            pw = lp.tile([tnr, W], F32, tag="pw")
                            nc.scalar.activation(pw[:], lnv[:], ACT.Exp, scale=1.0 / 2.4)
                            nc.vector.tensor_scalar(pw[:], pw[:], 1.055, 0.055, OP.mult, OP.subtract)
                            nc.vector.select(rgb[:, ci, :], maskg[:], ling[:], pw[:])
                        nc.sync.dma_start(
                            sc["rgb"][:, tr0 : tr0 + tnr, 1 : 1 + W].rearrange("c r w -> r c w").bitcast(F32),
                            rgb[:],
                        )

                # ---- conv1: K=27 im2col, 1 matmul per output row ----
                with (
                    tc.tile_pool(name="cv1", bufs=2) as p1,
                    tc.tile_pool(name="ps1", bufs=2, space="PSUM") as pp1,
                ):
                    rgbt = sc["rgb"]
                    for a, B in _bands(1, 267, 16):
                        im = p1.tile([27, B, W], F32R, tag="im")
                        for c in range(3):
                            for dy in range(3):
                                src_ap = bass.AP(
                                    rgbt[:].tensor,
                                    c * (RSLAB * WP) + (a - 1 + dy) * WP,
                                    [[1, 3], [WP, B], [1, W]],
                                )
                                p0 = 9 * c + 3 * dy
                                nc.sync.dma_start(im[p0 : p0 + 3, :, :], src_ap)
                        obuf = p1.tile([64, B, W], BF16, tag="obuf")
                        for j0 in range(0, B, 4):
                            g = min(4, B - j0)
                            ps = pp1.tile([64, 4, W], F32, tag="ps")
                            for j in range(j0, j0 + g):
                                nc.tensor.matmul(
                                    ps[:, j - j0, :],
                                    w1s[:],
                                    im[:, j, :],
                                    start=True,
                                    stop=True,
                                )
                            nc.scalar.activation(
                                obuf[:, j0 : j0 + g, :].rearrange("p r w -> p (r w)"),
                                ps[:, 0:g, :].rearrange("p r w -> p (r w)"),
                                ACT.Relu,
                                bias=b1s[0:64, :],
                            )
                        nc.sync.dma_start(sc["c1"][:, a : a + B, 1 : 1 + W], obuf[:])

                # ---- conv2: dy-pair packed K=128 (+ dy=2 K=64), fused maxpool ----
                with (
                    tc.tile_pool(name="cv2", bufs=2) as p2,
                    tc.tile_pool(name="ps2", bufs=2, space="PSUM") as pp2,
                ):
                    for a, B in _bands(2, 266, 16):
                        # parts 0:64 = c1 rows [a-1, a+B+1); parts 64:128 = rows [a, a+B+2)
                        inb = p2.tile([128, B + 2, WP], BF16, tag="inb")
                        nc.sync.dma_start(inb[0:64, :, :], sc["c1"][:, a - 1 : a + B + 1, :])
                        nc.sync.dma_start(inb[64:128, :, :], sc["c1"][:, a : a + B + 2, :])
                        obuf = p2.tile([64, B, W], F32, tag="obuf2")
                        for j0 in range(0, B, 4):
                            g = min(4, B - j0)
                            ps = pp2.tile([64, 4, W], F32, tag="ps2")
                            for j in range(j0, j0 + g):
                                for dx in range(3):
                                    nc.tensor.matmul(
                                        ps[:, j - j0, :],
                                        w2ps[:, dx, :],
                                        inb[:, j, dx : dx + W],
                                        start=(dx == 0),
                                        stop=False,
                                    )
                                for dx in range(3):
                                    nc.tensor.matmul(
                                        ps[:, j - j0, :],
                                        w2rs[:, dx, :],
                                        inb[0:64, j + 2, dx : dx + W],
                                        start=False,
                                        stop=(dx == 2),
                                    )
                            nc.scalar.activation(
                                obuf[:, j0 : j0 + g, :].rearrange("p r w -> p (r w)"),
                                ps[:, 0:g, :].rearrange("p r w -> p (r w)"),
                                ACT.Relu,
                                bias=b2s[0:64, :],
                            )
                        # fused maxpool 2x2 on this band (a is even, B even)
                        rview = obuf[:].rearrange("c (r two) w -> c r two w", two=2)
                        tmp = p2.tile([64, B // 2, W], F32, tag="mptmp")
                        nc.vector.tensor_max(tmp[:], rview[:, :, 0, :], rview[:, :, 1, :])
                        cview = tmp[:].rearrange("c r (x two) -> c r x two", two=2)
                        ob = p2.tile([64, B // 2, W2], BF16, tag="mpout")
                        nc.vector.tensor_max(ob[:], cview[:, :, :, 0], cview[:, :, :, 1])
                        nc.sync.dma_start(
                            sc["pl"][:, a // 2 : a // 2 + B // 2, 1 : 1 + W2], ob[:]
                        )

                # ---- conv3: K=64 -> 128, 2 rows per matmul ----
                with (
                    tc.tile_pool(name="cv3", bufs=2) as p3,
                    tc.tile_pool(name="ps3", bufs=2, space="PSUM") as pp3,
                ):
                    for a, B in _bands(2, 132, 16):
                        # parts 0:64 = pl rows [a-1, a+B+1); parts 64:128 = rows [a, a+B+2)
                        inb = p3.tile([128, B + 2, W2P], BF16, tag="inb3")
                        nc.sync.dma_start(inb[0:64, :, :], sc["pl"][:, a - 1 : a + B + 1, :])
                        nc.sync.dma_start(inb[64:128, :, :], sc["pl"][:, a : a + B + 2, :])
                        obuf = p3.tile([128, B, W2], BF16, tag="obuf3")
                        npair = B // 2
                        for g0 in range(0, npair, 4):
                            ng = min(4, npair - g0)
                            ps = pp3.tile([128, 4, 2, W2], F32, tag="ps3")
                            for g in range(g0, g0 + ng):
                                for dx in range(3):
                                    nc.tensor.matmul(
                                        ps[:, g - g0, :, :],
                                        w3ps[:, dx, :],
                                        inb[:, 2 * g : 2 * g + 2, dx : dx + W2],
                                        start=(dx == 0),
                                        stop=False,
                                    )
                                for dx in range(3):
                                    nc.tensor.matmul(
                                        ps[:, g - g0, :, :],
                                        w3rs[:, dx, :],
                                        inb[0:64, 2 * g + 2 : 2 * g + 4, dx : dx + W2],
                                        start=False,
                                        stop=(dx == 2),
                                    )
                            nc.scalar.activation(
                                obuf[:, 2 * g0 : 2 * g0 + 2 * ng, :].rearrange("p r w -> p (r w)"),
                                ps[:, 0:ng, :, :].rearrange("p r two w -> p (r two w)"),
                                ACT.Relu,
                                bias=b3s[:],
                            )
                        nc.sync.dma_start(sc["c3"][:, a : a + B, 1 : 1 + W2], obuf[:])

                # ---- conv4 (+ store pf / diff-square-accumulate) ----
                with (
                    tc.tile_pool(name="cv4", bufs=2) as p4,
                    tc.tile_pool(name="ps4", bufs=2, space="PSUM") as pp4,
                ):
                    for bi, (a, B) in enumerate(_bands(3, 131, 16)):
                        inb = p4.tile([128, B + 2, W2P], BF16, tag="inb4")
                        nc.sync.dma_start(inb[:], sc["c3"][:, a - 1 : a + B + 1, :])
                        if s == "P":
                            pfb = p4.tile([128, B, W2], F32, tag="pfb")
                        else:
                            pfb = p4.tile([128, B, W2], F32, tag="pfb")
                            nc.sync.dma_start(pfb[:], pf[:, a - 3 : a - 3 + B, :])
                        npair = B // 2
                        for g0 in range(0, npair, 4):
                            ng = min(4, npair - g0)
                            ps = pp4.tile([128, 4, 2, W2], F32, tag="ps4")
                            for g in range(g0, g0 + ng):
                                for t9 in range(9):
                                    dy, dx = divmod(t9, 3)
                                    nc.tensor.matmul(
                                        ps[:, g - g0, :, :],
                                        w4s[:, t9, :],
                                        inb[:, 2 * g + dy : 2 * g + dy + 2, dx : dx + W2],
                                        start=(t9 == 0),
                                        stop=(t9 == 8),
                                    )
                            ps_flat = ps[:, 0:ng, :, :].rearrange("p r two w -> p (r two w)")
                            if s == "P":
                                nc.scalar.copy(
                                    pfb[:, 2 * g0 : 2 * g0 + 2 * ng, :].rearrange("p r w -> p (r w)"),
                                    ps_flat,
                                )
                            else:
                                d = p4.tile([128, 8, W2], F32, tag="d4")
                                d_flat = d[:, 0 : 2 * ng, :].rearrange("p r w -> p (r w)")
                                nc.vector.scalar_tensor_tensor(
                                    d_flat,
                                    ps_flat,
                                    0.0,
                                    pfb[:, 2 * g0 : 2 * g0 + 2 * ng, :].rearrange("p r w -> p (r w)"),
                                    OP.add,
                                    OP.subtract,
                                )
                                d2 = p4.tile([128, 8, W2], F32, tag="d42")
                                slot = SLOT_PERC + 2 * bi + g0 // 4
                                nc.scalar.activation(
                                    d2[:, 0 : 2 * ng, :].rearrange("p r w -> p (r w)"),
                                    d_flat,
                                    ACT.Square,
                                    accum_out=acc[:, slot : slot + 1],
                                )
                        if s == "P":
                            nc.sync.dma_start(pf[:, a - 3 : a - 3 + B, :], pfb[:])

            # =========== MSE ===========
            with tc.tile_pool(name="mse", bufs=2) as pmse:
                for ti, r0 in enumerate((6, 134)):
                    pt = pmse.tile([128, 2, W], F32, tag="msep")
                    nc.sync.dma_start(
                        pt[:], pred[:, r0 : r0 + 128, :].rearrange("c r w -> r c w")
                    )
                    tt = pmse.tile([128, 2, W], F32, tag="mset")
                    nc.sync.dma_start(
                        tt[:], targ[:, r0 : r0 + 128, :].rearrange("c r w -> r c w")
                    )
                    d = pmse.tile([128, 2, W], F32, tag="msed")
                    nc.vector.scalar_tensor_tensor(d[:], pt[:], 0.0, tt[:], OP.add, OP.subtract)
                    d2 = pmse.tile([128, 2, W], F32, tag="msed2")
                    nc.scalar.activation(
                        d2[:].rearrange("p c w -> p (c w)"),
                        d[:].rearrange("p c w -> p (c w)"),
                        ACT.Square,
                        accum_out=acc[:, SLOT_MSE + ti : SLOT_MSE + ti + 1],
                    )

            # =========== SSIM ===========
            with (
                tc.tile_pool(name="ssim", bufs=1) as ps_,
                tc.tile_pool(name="ssimw", bufs=2) as pw_,
                tc.tile_pool(name="pssim", bufs=5, space="PSUM") as pps,
            ):
                for ch in range(2):
                    tiles = {}
                    tdefs = ((0, 1, 128), (1, 129, 128), (2, 257, 10))
                    for tid, tr0, tnr in tdefs:
                        mp_ = {}
                        mp_["p"] = ps_.tile([tnr, W], F32, tag=f"sp{tid}", name=f"sp{tid}")
                        nc.sync.dma_start(mp_["p"][:], pred[ch, tr0 : tr0 + tnr, :])
                        mp_["t"] = ps_.tile([tnr, W], F32, tag=f"st{tid}", name=f"st{tid}")
                        nc.sync.dma_start(mp_["t"][:], targ[ch, tr0 : tr0 + tnr, :])
                        mp_["pp"] = ps_.tile([tnr, W], F32, tag=f"spp{tid}", name=f"spp{tid}")
                        nc.vector.tensor_mul(mp_["pp"][:], mp_["p"][:], mp_["p"][:])
                        mp_["tt"] = ps_.tile([tnr, W], F32, tag=f"stt{tid}", name=f"stt{tid}")
                        nc.vector.tensor_mul(mp_["tt"][:], mp_["t"][:], mp_["t"][:])
                        mp_["pt"] = ps_.tile([tnr, W], F32, tag=f"spt{tid}", name=f"spt{tid}")
                        nc.vector.tensor_mul(mp_["pt"][:], mp_["p"][:], mp_["t"][:])
                        tiles[tid] = mp_
                    for oi in range(2):
                        # O0 <- bandA@T0 + bandB@T1 ; O1 <- bandA@T1 + bandB[0:10]@T2
                        srcs = ((0, bAs[:], 128), (1, bBs[:], 128)) if oi == 0 else (
                            (1, bAs[:], 128), (2, bBs[0:10, :], 10))
                        vs = {}
                        for mname in ("p", "t", "pp", "tt", "pt"):
                            psv = pps.tile([128, W], F32, tag="psv")
                            for si_, (tid, band_ap, kk) in enumerate(srcs):
                                nc.tensor.matmul(
                                    psv[:],
                                    band_ap,
                                    tiles[tid][mname][:],
                                    start=(si_ == 0),
                                    stop=(si_ == 1),
                                )
                            wb = pw_.tile([128, 522], F32, tag=f"wb{mname}")
                            pad = bass.AP(wb[:].tensor, wb[:].offset, [[522, 128], [517, 2], [1, 5]])
                            nc.vector.memset(pad, 0.0)
                            nc.scalar.copy(wb[:, 5:517], psv[:])
                            # horizontal 11-tap sliding sum via log-shifts
                            s2 = pw_.tile([128, 522], F32, tag="s2")
                            nc.vector.tensor_add(s2[:, 0:521], wb[:, 0:521], wb[:, 1:522])
                            s3 = pw_.tile([128, 522], F32, tag="s3")
                            nc.vector.tensor_add(s3[:, 0:520], s2[:, 0:520], wb[:, 2:522])
                            s4 = pw_.tile([128, 522], F32, tag="s4")
                            nc.vector.tensor_add(s4[:, 0:517], s2[:, 0:517], s2[:, 2:519])
                            s8 = pw_.tile([128, 522], F32, tag="s8")
                            nc.vector.tensor_add(s8[:, 0:513], s4[:, 0:513], s4[:, 4:517])
                            sv = pw_.tile([128, W], F32, tag=f"sv{mname}")
                            nc.vector.tensor_add(sv[:], s8[:, 0:512], s3[:, 8:520])
                            vs[mname] = sv
                        # SSIM formula from window sums (mu = s/121)
                        A = pw_.tile([128, W], F32, tag="fA")
                        nc.vector.tensor_mul(A[:], vs["p"][:], vs["t"][:])
                        num1 = pw_.tile([128, W], F32, tag="fnum1")
                        nc.vector.tensor_scalar(num1[:], A[:], 2.0 * INV121SQ, C1, OP.mult, OP.add)
                        t1 = pw_.tile([128, W], F32, tag="ft1")
                        nc.vector.tensor_scalar(t1[:], vs["pt"][:], 2.0 * INV121, C2, OP.mult, OP.add)
                        num2 = pw_.tile([128, W], F32, tag="fnum2")
                        nc.vector.scalar_tensor_tensor(num2[:], A[:], -2.0 * INV121SQ, t1[:], OP.mult, OP.add)
                        sq1 = pw_.tile([128, W], F32, tag="fsq1")
                        nc.vector.tensor_mul(sq1[:], vs["p"][:], vs["p"][:])
                        sq2 = pw_.tile([128, W], F32, tag="fsq2")
                        nc.vector.tensor_mul(sq2[:], vs["t"][:], vs["t"][:])
                        ssum = pw_.tile([128, W], F32, tag="fssum")
                        nc.vector.tensor_add(ssum[:], sq1[:], sq2[:])
                        den1 = pw_.tile([128, W], F32, tag="fden1")
                        nc.vector.tensor_scalar(den1[:], ssum[:], INV121SQ, C1, OP.mult, OP.add)
                        u = pw_.tile([128, W], F32, tag="fu")
                        nc.vector.tensor_add(u[:], vs["pp"][:], vs["tt"][:])
                        u2 = pw_.tile([128, W], F32, tag="fu2")
                        nc.vector.tensor_scalar(u2[:], u[:], INV121, C2, OP.mult, OP.add)
                        den2 = pw_.tile([128, W], F32, tag="fden2")
                        nc.vector.scalar_tensor_tensor(den2[:], ssum[:], -INV121SQ, u2[:], OP.mult, OP.add)
                        num = pw_.tile([128, W], F32, tag="fnum")
                        nc.vector.tensor_mul(num[:], num1[:], num2[:])
                        den = pw_.tile([128, W], F32, tag="fden")
                        nc.vector.tensor_mul(den[:], den1[:], den2[:])
                        rden = pw_.tile([128, W], F32, tag="frden")
                        nc.vector.reciprocal(rden[:], den[:])
                        smap = pw_.tile([128, W], F32, tag="fsmap")
                        slot = SLOT_SSIM + 2 * ch + oi
                        nc.vector.scalar_tensor_tensor(
                            smap[:], num[:], 0.0, rden[:], OP.add, OP.mult,
                            accum_out=acc[:, slot : slot + 1],
                        )

            nc.sync.dma_start(parts_out[:], acc[:])

    _legalize_waits(nc)
    return nc, ins


_CACHE = {}


def _get_program():
    if "nc" not in _CACHE:
        _CACHE["nc"] = build_program()
    return _CACHE["nc"]


def _host_inputs(pred, target, w1, b1, w2, b2, w3, b3, w4, b4):
    """Build the 8 per-core input maps."""
    w1i = np.ascontiguousarray(np.transpose(w1, (1, 2, 3, 0)).reshape(27, 64))
    wt2 = np.transpose(w2, (2, 3, 1, 0))  # [dy, dx, c, o]
    w2p_h = np.ascontiguousarray(
        np.stack([np.concatenate([wt2[0, dx], wt2[1, dx]], axis=0) for dx in range(3)])
    ).astype(ml_dtypes.bfloat16)
    w2r_h = np.ascontiguousarray(np.stack([wt2[2, dx] for dx in range(3)])).astype(ml_dtypes.bfloat16)
    wt3 = np.transpose(w3, (2, 3, 1, 0))
    w3p_h = np.ascontiguousarray(
        np.stack([np.concatenate([wt3[0, dx], wt3[1, dx]], axis=0) for dx in range(3)])
    ).astype(ml_dtypes.bfloat16)
    w3r_h = np.ascontiguousarray(np.stack([wt3[2, dx] for dx in range(3)])).astype(ml_dtypes.bfloat16)
    w4t = np.ascontiguousarray(np.transpose(w4, (2, 3, 1, 0)).reshape(9, 128, 128)).astype(ml_dtypes.bfloat16)
    b1d = np.concatenate([b1, b1]).reshape(128, 1).astype(np.float32)
    b2d = np.concatenate([b2, b2]).reshape(128, 1).astype(np.float32)
    b3d = b3.reshape(128, 1).astype(np.float32)
    k = np.arange(128)[:, None]
    m = np.arange(128)[None, :]
    bandA = ((k >= m) & (k < m + 11)).astype(np.float32)
    bandB = ((k >= m - 128) & (k < m - 117)).astype(np.float32)

    in_maps = []
    for c in range(8):
        b, h = divmod(c, 2)
        g0 = 256 * h - 6
        maps = {}
        for nm, full in (("pred", pred), ("targ", target)):
            slab = np.zeros((2, RSLAB, W), np.float32)
            lo, hi = max(0, g0), min(512, g0 + RSLAB)
            slab[:, lo - g0 : hi - g0, :] = full[b][:, lo:hi, :]
            maps[nm] = slab
        maps.update(
            w1i=w1i, w2p=w2p_h, w2r=w2r_h, w3p=w3p_h, w3r=w3r_h, w4t=w4t,
            b1d=b1d, b2d=b2d, b3d=b3d, bandA=bandA, bandB=bandB,
        )
        in_maps.append(maps)
    return in_maps


def _combine(results):
    mse_s = 0.0
    ssim_s = 0.0
    perc_s = 0.0
    for r in results:
        p = r["parts"].astype(np.float64)
        mse_s += p[:, SLOT_MSE : SLOT_MSE + 2].sum()
        ssim_s += p[:, SLOT_SSIM : SLOT_SSIM + 4].sum()
        perc_s += p[:, SLOT_PERC : SLOT_PERC + 16].sum()
    n_px = 4 * 2 * 512 * 512
    n_pc = 4 * 128 * 256 * 256
    loss = (
        mse_s / n_px
        + 0.2 * (perc_s / n_pc)
        + 0.2 * (1.0 - ssim_s / n_px)
    )
    return np.float32(loss)


def kernel(pred, target, w1, b1, w2, b2, w3, b3, w4, b4, _trace=False):
    nc, _ = _get_program()
    in_maps = _host_inputs(pred, target, w1, b1, w2, b2, w3, b3, w4, b4)
    res = run_bass_kernel_spmd(nc, in_maps, core_ids=list(range(8)), trace=_trace)
    out = _combine(res.results)
    if _trace:
        return out, res
    return out
